# revision 2
# baseline (speedup 1.0000x reference)
"""Heat-kernel graph diffusion on 8 Trainium2 NeuronCores.

Computes out = expm(-t*L) @ x for a graph Laplacian L [2048,2048] and node
features x [2048,512], t scalar.

Method: Chebyshev expansion of exp(-t*lam) on [0, lam_b] applied to the
action on x (no dense expm):
    out = sum_k c_k T_k(M) x,   M = (2/lam_b) L - I,
    c_0 = e^{-a} I_0(a), c_k = 2 e^{-a} (-1)^k I_k(a),  a = t*lam_b/2,
with lam_b = 2*max(diag(L)) (Gershgorin bound for a Laplacian; always
>= lam_max). K ~ 20 terms for t=0.5. Bessel I_k via Miller's backward
recurrence (pure numpy, no scipy).

Sharding: x column-sharded 8 ways (64 channels/core), L replicated; the
recurrence is embarrassingly parallel across channels - no collectives.

Device kernel (per core, natural layout [node, ch]):
  - L is exactly representable in bf16 (entries are multiples of 0.5 < 256),
    so it is passed pre-cast to bf16 and used as 128x128 stationary matmul
    weights (full PE array, 1 cyc/row). If a pathological L is not bf16-exact,
    a second bf16 matrix L_lo = L - bf16(L) is also multiplied in.
  - fp32 state y is split per term into bf16 hi+lo halves, concatenated as a
    [128, 128] moving operand; PSUM accumulates z_hi|z_lo in fp32.
  - Chebyshev recurrence y_next = 2a*(L y) - 2 y - y_prev and accumulation
    run in fp32 on the Vector/Scalar engines.
Measured end-to-end relative error vs the fp64 reference path: ~3e-5.
"""

import functools
import math

import numpy as np
import ml_dtypes

import concourse.bacc as bacc
import concourse.mybir as mybir
import concourse.tile as tile
from concourse.bass_utils import run_bass_kernel_spmd

N = 2048
D = 512
NCORES = 8
DSH = D // NCORES      # 64 channels per core
P = 128                # partitions
KB = N // P            # 16 contraction blocks
IB = N // P            # 16 output-row blocks
COEF_TOL = 1e-7
KMAX = 280

BF16 = np.dtype(ml_dtypes.bfloat16)


def _bessel_ive(nmax, a):
    """e^{-a} I_k(a), k=0..nmax, via Miller's backward recurrence (float64)."""
    if a < 1e-12:
        out = np.zeros(nmax + 1)
        out[0] = 1.0
        return out
    m = int(max(nmax, a) + 40 + 2 * math.sqrt(max(nmax, a)))
    r = np.zeros(m + 2)
    r[m] = 1e-300
    for k in range(m, 0, -1):
        r[k - 1] = r[k + 1] + (2.0 * k / a) * r[k]
        if r[k - 1] > 1e250:
            r /= r[k - 1]
    s = r[0] + 2.0 * np.sum(r[1:m + 1])
    return r[: nmax + 1] / s


def _cheb_coeffs(t, lam_b, tol=COEF_TOL, kcap=KMAX):
    a = t * lam_b / 2.0
    iv = _bessel_ive(kcap, a)
    c = np.empty(kcap + 1)
    c[0] = iv[0]
    c[1:] = 2.0 * iv[1:] * ((-1.0) ** np.arange(1, kcap + 1))
    keep = np.nonzero(np.abs(c) > tol)[0]
    K = max(1, int(keep[-1]) if len(keep) else 1)
    return c[: K + 1]


@functools.lru_cache(maxsize=4)
def _build(coeffs_key, alpha, use_llo):
    """Compile the per-core NEFF. coeffs_key: tuple of per-term float coeffs."""
    c = np.array(coeffs_key, dtype=np.float64)
    K = len(c) - 1
    f32 = mybir.dt.float32
    bf16 = mybir.dt.bfloat16

    nc = bacc.Bacc("TRN2", target_bir_lowering=False, debug=False,
                   num_devices=NCORES)
    L_d = nc.dram_tensor("L", [N, N], bf16, kind="ExternalInput").ap()
    Llo_d = None
    if use_llo:
        Llo_d = nc.dram_tensor("Llo", [N, N], bf16, kind="ExternalInput").ap()
    x_d = nc.dram_tensor("x", [N, DSH], f32, kind="ExternalInput").ap()
    o_d = nc.dram_tensor("out", [N, DSH], f32, kind="ExternalOutput").ap()

    with tile.TileContext(nc) as tc:
        with tc.tile_pool(name="big", bufs=1) as big, \
             tc.tile_pool(name="state", bufs=1) as state, \
             tc.tile_pool(name="psum", bufs=2, space="PSUM") as psum:
            L_sb = big.tile([P, KB, N], bf16, tag="L")
            nc.sync.dma_start(out=L_sb, in_=L_d.rearrange("(k p) n -> p k n", p=P))
            if use_llo:
                Llo_sb = big.tile([P, KB, N], bf16, tag="Llo")
                nc.sync.dma_start(out=Llo_sb,
                                  in_=Llo_d.rearrange("(k p) n -> p k n", p=P))
            x_sb = state.tile([P, KB, DSH], f32, tag="x")
            nc.sync.dma_start(out=x_sb, in_=x_d.rearrange("(k p) c -> p k c", p=P))

            # state buffers (rotating) + accumulator + scratch
            ys = [state.tile([P, KB, DSH], f32, tag=f"y{i}", name=f"y{i}")
                  for i in range(3)]
            acc = state.tile([P, KB, DSH], f32, tag="acc")
            zh = state.tile([P, KB, DSH], f32, tag="zh")
            u = state.tile([P, KB, DSH], f32, tag="u")
            rhs_cat = state.tile([P, KB, 2 * DSH], bf16, tag="cat")

            sub = mybir.AluOpType.subtract
            add = mybir.AluOpType.add
            mult = mybir.AluOpType.mult

            def split_into_cat(src):
                """rhs_cat <- [bf16(src) | bf16(src - hi)]"""
                hi = rhs_cat[:, :, 0:DSH]
                lo = rhs_cat[:, :, DSH:2 * DSH]
                nc.scalar.copy(out=hi, in_=src)
                # lo = (hi * -1) + src
                nc.vector.scalar_tensor_tensor(out=lo, in0=hi, scalar=-1.0,
                                               in1=src, op0=mult, op1=add)

            # y0 = x; acc = c0 * x
            nc.vector.tensor_copy(out=ys[0], in_=x_sb)
            nc.vector.tensor_scalar_mul(acc, x_sb, float(c[0]))
            split_into_cat(ys[0])

            for k in range(1, K + 1):
                scale = float(2.0 * alpha) if k >= 2 else float(alpha)
                ps = psum.tile([P, IB, 2 * DSH], f32, tag="ps")
                for ib in range(IB):
                    for kb in range(KB):
                        nc.tensor.matmul(
                            ps[:, ib, :],
                            L_sb[:, kb, ib * P:(ib + 1) * P],
                            rhs_cat[:, kb, :],
                            start=(kb == 0),
                            stop=(kb == KB - 1 and not use_llo),
                        )
                    if use_llo:
                        for kb in range(KB):
                            nc.tensor.matmul(
                                ps[:, ib, 0:DSH],
                                Llo_sb[:, kb, ib * P:(ib + 1) * P],
                                rhs_cat[:, kb, 0:DSH],
                                start=False,
                                stop=(kb == KB - 1),
                            )
                # zh = scale * ps_hi   (ACT, one PSUM read)
                nc.scalar.mul(zh, ps[:, :, 0:DSH], scale)
                # u = (ps_lo * scale) + zh = scale * (L y)      [= a*z or 2a*z]
                nc.vector.scalar_tensor_tensor(out=u, in0=ps[:, :, DSH:2 * DSH],
                                               scalar=scale, in1=zh,
                                               op0=mult, op1=add)
                y_cur = ys[(k - 1) % 3]
                y_next = ys[k % 3]
                if k == 1:
                    # y1 = a*z - y0
                    nc.vector.scalar_tensor_tensor(out=y_next, in0=y_cur,
                                                   scalar=-1.0, in1=u,
                                                   op0=mult, op1=add)
                else:
                    y_prev = ys[(k - 2) % 3]
                    # u2 = u - 2*y_cur ; y_next = u2 - y_prev
                    nc.vector.scalar_tensor_tensor(out=u, in0=y_cur,
                                                   scalar=-2.0, in1=u,
                                                   op0=mult, op1=add)
                    nc.vector.scalar_tensor_tensor(out=y_next, in0=y_prev,
                                                   scalar=-1.0, in1=u,
                                                   op0=mult, op1=add)
                # acc += c_k * y_next
                nc.vector.scalar_tensor_tensor(out=acc, in0=y_next,
                                               scalar=float(c[k]), in1=acc,
                                               op0=mult, op1=add)
                if k < K:
                    split_into_cat(y_next)

            nc.sync.dma_start(out=o_d.rearrange("(k p) c -> p k c", p=P), in_=acc)

    nc.compile()
    return nc


def kernel(x, L, t):
    x = np.ascontiguousarray(np.asarray(x, dtype=np.float32))
    L = np.ascontiguousarray(np.asarray(L, dtype=np.float32))
    tv = float(max(float(np.asarray(t, dtype=np.float32)), 1e-8))
    assert x.shape == (N, D) and L.shape == (N, N)

    lam_b = max(2.0 * float(np.diagonal(L).max()), 1e-6)
    alpha = 2.0 / lam_b
    c = _cheb_coeffs(tv, lam_b)

    L_hi = L.astype(BF16)
    L_res = L - L_hi.astype(np.float32)
    use_llo = bool(np.any(L_res != 0.0))

    nc = _build(tuple(float(v) for v in c), float(alpha), use_llo)

    in_maps = []
    for core in range(NCORES):
        m = {"L": L_hi, "x": np.ascontiguousarray(x[:, core * DSH:(core + 1) * DSH])}
        if use_llo:
            m["Llo"] = L_res.astype(BF16)
        in_maps.append(m)

    res = run_bass_kernel_spmd(nc, in_maps, core_ids=list(range(NCORES)))
    out = np.empty((N, D), dtype=np.float32)
    for core in range(NCORES):
        out[:, core * DSH:(core + 1) * DSH] = res.results[core]["out"]
    kernel.last_exec_time_ns = res.exec_time_ns
    return out


kernel.last_exec_time_ns = None


# revision 3
# speedup vs baseline: 1.0134x; 1.0134x over previous
"""Heat-kernel graph diffusion on 8 Trainium2 NeuronCores.

Computes out = expm(-t*L) @ x for a graph Laplacian L [2048,2048] and node
features x [2048,512], t scalar.

Method: Chebyshev expansion of exp(-t*lam) on [0, lam_b] applied to the
action on x (no dense expm):
    out = sum_k c_k T_k(M) x,   M = (2/lam_b) L - I,
    c_0 = e^{-a} I_0(a), c_k = 2 e^{-a} (-1)^k I_k(a),  a = t*lam_b/2,
with lam_b = 2*max(diag(L)) (Gershgorin bound for a Laplacian; always
>= lam_max). K ~ 20 terms for t=0.5. Bessel I_k via Miller's backward
recurrence (pure numpy, no scipy).

Sharding: x column-sharded 8 ways (64 channels/core), L replicated; the
recurrence is embarrassingly parallel across channels - no collectives.

Device kernel (per core, natural layout [node, ch]):
  - L is exactly representable in bf16 (entries are multiples of 0.5 < 256),
    so it is passed pre-cast to bf16 and used as 128x128 stationary matmul
    weights (full PE array, 1 cyc/row). If a pathological L is not bf16-exact,
    a second bf16 matrix L_lo = L - bf16(L) is also multiplied in.
  - fp32 state y is split per term into bf16 hi+lo halves, concatenated as a
    [128, 128] moving operand; PSUM accumulates z_hi|z_lo in fp32.
  - Chebyshev recurrence y_next = 2a*(L y) - 2 y - y_prev and accumulation
    run in fp32 on the Vector/Scalar engines.
Measured end-to-end relative error vs the fp64 reference path: ~3e-5.
"""

import functools
import math

import numpy as np
import ml_dtypes

import concourse.bacc as bacc
import concourse.mybir as mybir
import concourse.tile as tile
from concourse.bass_utils import run_bass_kernel_spmd

N = 2048
D = 512
NCORES = 8
DSH = D // NCORES      # 64 channels per core
P = 128                # partitions
KB = N // P            # 16 contraction blocks
IB = N // P            # 16 output-row blocks
COEF_TOL = 1e-7
KMAX = 280

BF16 = np.dtype(ml_dtypes.bfloat16)


def _bessel_ive(nmax, a):
    """e^{-a} I_k(a), k=0..nmax, via Miller's backward recurrence (float64)."""
    if a < 1e-12:
        out = np.zeros(nmax + 1)
        out[0] = 1.0
        return out
    m = int(max(nmax, a) + 40 + 2 * math.sqrt(max(nmax, a)))
    r = np.zeros(m + 2)
    r[m] = 1e-300
    for k in range(m, 0, -1):
        r[k - 1] = r[k + 1] + (2.0 * k / a) * r[k]
        if r[k - 1] > 1e250:
            r /= r[k - 1]
    s = r[0] + 2.0 * np.sum(r[1:m + 1])
    return r[: nmax + 1] / s


def _cheb_coeffs(t, lam_b, tol=COEF_TOL, kcap=KMAX):
    a = t * lam_b / 2.0
    iv = _bessel_ive(kcap, a)
    c = np.empty(kcap + 1)
    c[0] = iv[0]
    c[1:] = 2.0 * iv[1:] * ((-1.0) ** np.arange(1, kcap + 1))
    keep = np.nonzero(np.abs(c) > tol)[0]
    K = max(1, int(keep[-1]) if len(keep) else 1)
    return c[: K + 1]


@functools.lru_cache(maxsize=4)
def _build(coeffs_key, alpha, use_llo):
    """Compile the per-core NEFF. coeffs_key: tuple of per-term float coeffs."""
    c = np.array(coeffs_key, dtype=np.float64)
    K = len(c) - 1
    f32 = mybir.dt.float32
    bf16 = mybir.dt.bfloat16

    nc = bacc.Bacc("TRN2", target_bir_lowering=False, debug=False,
                   num_devices=NCORES)
    L_d = nc.dram_tensor("L", [N, N], bf16, kind="ExternalInput").ap()
    Llo_d = None
    if use_llo:
        Llo_d = nc.dram_tensor("Llo", [N, N], bf16, kind="ExternalInput").ap()
    x_d = nc.dram_tensor("x", [N, DSH], f32, kind="ExternalInput").ap()
    o_d = nc.dram_tensor("out", [N, DSH], f32, kind="ExternalOutput").ap()

    with tile.TileContext(nc) as tc:
        with tc.tile_pool(name="big", bufs=1) as big, \
             tc.tile_pool(name="state", bufs=1) as state, \
             tc.tile_pool(name="psum", bufs=2, space="PSUM") as psum:
            L_sb = big.tile([P, KB, N], bf16, tag="L")
            nc.sync.dma_start(out=L_sb, in_=L_d.rearrange("(k p) n -> p k n", p=P))
            if use_llo:
                Llo_sb = big.tile([P, KB, N], bf16, tag="Llo")
                nc.sync.dma_start(out=Llo_sb,
                                  in_=Llo_d.rearrange("(k p) n -> p k n", p=P))
            x_sb = state.tile([P, KB, DSH], f32, tag="x")
            nc.sync.dma_start(out=x_sb, in_=x_d.rearrange("(k p) c -> p k c", p=P))

            # state buffers (rotating) + accumulator + scratch
            ys = [state.tile([P, KB, DSH], f32, tag=f"y{i}", name=f"y{i}")
                  for i in range(3)]
            acc = state.tile([P, KB, DSH], f32, tag="acc")
            zh = state.tile([P, KB, DSH], f32, tag="zh")
            u = state.tile([P, KB, DSH], f32, tag="u")
            rhs_cat = state.tile([P, KB, 2 * DSH], bf16, tag="cat")

            sub = mybir.AluOpType.subtract
            add = mybir.AluOpType.add
            mult = mybir.AluOpType.mult

            def split_into_cat(src):
                """rhs_cat <- [bf16(src) | bf16(src - hi)]"""
                hi = rhs_cat[:, :, 0:DSH]
                lo = rhs_cat[:, :, DSH:2 * DSH]
                nc.scalar.copy(out=hi, in_=src)
                # lo = (hi * -1) + src
                nc.vector.scalar_tensor_tensor(out=lo, in0=hi, scalar=-1.0,
                                               in1=src, op0=mult, op1=add)

            # y0 = x; acc = c0 * x
            nc.vector.tensor_copy(out=ys[0], in_=x_sb)
            nc.vector.tensor_scalar_mul(acc, x_sb, float(c[0]))
            split_into_cat(ys[0])

            for k in range(1, K + 1):
                scale = float(2.0 * alpha) if k >= 2 else float(alpha)
                ps = psum.tile([P, IB, 2 * DSH], f32, tag="ps")
                for ib in range(IB):
                    for kb in range(KB):
                        nc.tensor.matmul(
                            ps[:, ib, :],
                            L_sb[:, kb, ib * P:(ib + 1) * P],
                            rhs_cat[:, kb, :],
                            start=(kb == 0),
                            stop=(kb == KB - 1 and not use_llo),
                        )
                    if use_llo:
                        for kb in range(KB):
                            nc.tensor.matmul(
                                ps[:, ib, 0:DSH],
                                Llo_sb[:, kb, ib * P:(ib + 1) * P],
                                rhs_cat[:, kb, 0:DSH],
                                start=False,
                                stop=(kb == KB - 1),
                            )
                # zh = scale * ps_hi   (ACT, one PSUM read)
                nc.scalar.mul(zh, ps[:, :, 0:DSH], scale)
                # u = (ps_lo * scale) + zh = scale * (L y)      [= a*z or 2a*z]
                nc.vector.scalar_tensor_tensor(out=u, in0=ps[:, :, DSH:2 * DSH],
                                               scalar=scale, in1=zh,
                                               op0=mult, op1=add)
                y_cur = ys[(k - 1) % 3]
                y_next = ys[k % 3]
                if k == 1:
                    # y1 = a*z - y0
                    nc.vector.scalar_tensor_tensor(out=y_next, in0=y_cur,
                                                   scalar=-1.0, in1=u,
                                                   op0=mult, op1=add)
                else:
                    y_prev = ys[(k - 2) % 3]
                    # u2 = u - 2*y_cur ; y_next = u2 - y_prev
                    nc.vector.scalar_tensor_tensor(out=u, in0=y_cur,
                                                   scalar=-2.0, in1=u,
                                                   op0=mult, op1=add)
                    nc.vector.scalar_tensor_tensor(out=y_next, in0=y_prev,
                                                   scalar=-1.0, in1=u,
                                                   op0=mult, op1=add)
                # acc += c_k * y_next
                nc.vector.scalar_tensor_tensor(out=acc, in0=y_next,
                                               scalar=float(c[k]), in1=acc,
                                               op0=mult, op1=add)
                if k < K:
                    split_into_cat(y_next)

            nc.sync.dma_start(out=o_d.rearrange("(k p) c -> p k c", p=P), in_=acc)

    nc.compile()
    return nc


def kernel(x, L, t):
    x = np.ascontiguousarray(np.asarray(x, dtype=np.float32))
    L = np.ascontiguousarray(np.asarray(L, dtype=np.float32))
    tv = float(max(float(np.asarray(t, dtype=np.float32)), 1e-8))
    assert x.shape == (N, D) and L.shape == (N, N)

    lam_b = max(2.0 * float(np.diagonal(L).max()), 1e-6)
    alpha = 2.0 / lam_b
    c = _cheb_coeffs(tv, lam_b)

    L_hi = L.astype(BF16)
    L_res = L - L_hi.astype(np.float32)
    use_llo = bool(np.any(L_res != 0.0))

    nc = _build(tuple(float(v) for v in c), float(alpha), use_llo)

    in_maps = []
    for core in range(NCORES):
        m = {"L": L_hi, "x": np.ascontiguousarray(x[:, core * DSH:(core + 1) * DSH])}
        if use_llo:
            m["Llo"] = L_res.astype(BF16)
        in_maps.append(m)

    res = run_bass_kernel_spmd(nc, in_maps, core_ids=list(range(NCORES)))
    out = np.empty((N, D), dtype=np.float32)
    for core in range(NCORES):
        out[:, core * DSH:(core + 1) * DSH] = res.results[core]["out"]
    kernel.last_exec_time_ns = res.exec_time_ns
    kernel.last_results = res
    return out


kernel.last_exec_time_ns = None
kernel.last_results = None


# revision 5
# speedup vs baseline: 1.0730x; 1.0588x over previous
"""Heat-kernel graph diffusion on 8 Trainium2 NeuronCores.

Computes out = expm(-t*L) @ x for a graph Laplacian L [2048,2048] and node
features x [2048,512], t scalar.

Method: Chebyshev expansion of exp(-t*lam) on [0, lam_b] applied to the
action on x (no dense expm):
    out = sum_k c_k T_k(M) x,   M = (2/lam_b) L - I,
    c_0 = e^{-a} I_0(a), c_k = 2 e^{-a} (-1)^k I_k(a),  a = t*lam_b/2,
with lam_b = 2*max(diag(L)) (Gershgorin bound for a Laplacian; always
>= lam_max). K ~ 20 terms for t=0.5. Bessel I_k via Miller's backward
recurrence (pure numpy, no scipy).

Sharding: x column-sharded 8 ways (64 channels/core), L replicated; the
recurrence is embarrassingly parallel across channels - no collectives.

Device kernel (per core, natural layout [node, ch]):
  - L is exactly representable in bf16 (entries are multiples of 0.5 < 256),
    so it is passed pre-cast to bf16 and used as 128x128 stationary matmul
    weights (full PE array, 1 cyc/row). If a pathological L is not bf16-exact,
    a second bf16 matrix L_lo = L - bf16(L) is also multiplied in.
  - fp32 state y is split per term into bf16 hi+lo halves, concatenated as a
    [128, 128] moving operand; PSUM accumulates z_hi|z_lo in fp32.
  - Chebyshev recurrence y_next = 2a*(L y) - 2 y - y_prev and accumulation
    run in fp32 on the Vector/Scalar engines.
Measured end-to-end relative error vs the fp64 reference path: ~3e-5.
"""

import functools
import math

import numpy as np
import ml_dtypes

import concourse.bacc as bacc
import concourse.mybir as mybir
import concourse.tile as tile
from concourse.bass_utils import run_bass_kernel_spmd

N = 2048
D = 512
NCORES = 8
DSH = D // NCORES      # 64 channels per core
P = 128                # partitions
KB = N // P            # 16 contraction blocks
IB = N // P            # 16 output-row blocks
COEF_TOL = 1e-7
KMAX = 280

BF16 = np.dtype(ml_dtypes.bfloat16)


def _bessel_ive(nmax, a):
    """e^{-a} I_k(a), k=0..nmax, via Miller's backward recurrence (float64)."""
    if a < 1e-12:
        out = np.zeros(nmax + 1)
        out[0] = 1.0
        return out
    m = int(max(nmax, a) + 40 + 2 * math.sqrt(max(nmax, a)))
    r = np.zeros(m + 2)
    r[m] = 1e-300
    for k in range(m, 0, -1):
        r[k - 1] = r[k + 1] + (2.0 * k / a) * r[k]
        if r[k - 1] > 1e250:
            r /= r[k - 1]
    s = r[0] + 2.0 * np.sum(r[1:m + 1])
    return r[: nmax + 1] / s


def _cheb_coeffs(t, lam_b, tol=COEF_TOL, kcap=KMAX):
    a = t * lam_b / 2.0
    iv = _bessel_ive(kcap, a)
    c = np.empty(kcap + 1)
    c[0] = iv[0]
    c[1:] = 2.0 * iv[1:] * ((-1.0) ** np.arange(1, kcap + 1))
    keep = np.nonzero(np.abs(c) > tol)[0]
    K = max(1, int(keep[-1]) if len(keep) else 1)
    return c[: K + 1]


@functools.lru_cache(maxsize=4)
def _build(coeffs_key, alpha, use_llo):
    """Compile the per-core NEFF. coeffs_key: tuple of per-term float coeffs."""
    c = np.array(coeffs_key, dtype=np.float64)
    K = len(c) - 1
    f32 = mybir.dt.float32
    bf16 = mybir.dt.bfloat16

    nc = bacc.Bacc("TRN2", target_bir_lowering=False, debug=False,
                   num_devices=NCORES)
    L_d = nc.dram_tensor("L", [N, N], bf16, kind="ExternalInput").ap()
    Llo_d = None
    if use_llo:
        Llo_d = nc.dram_tensor("Llo", [N, N], bf16, kind="ExternalInput").ap()
    x_d = nc.dram_tensor("x", [N, DSH], f32, kind="ExternalInput").ap()
    o_d = nc.dram_tensor("out", [N, DSH], f32, kind="ExternalOutput").ap()

    with tile.TileContext(nc) as tc:
        with tc.tile_pool(name="big", bufs=1) as big, \
             tc.tile_pool(name="state", bufs=1) as state, \
             tc.tile_pool(name="psum", bufs=2, space="PSUM") as psum:
            x_sb = state.tile([P, KB, DSH], f32, tag="x")
            nc.sync.dma_start(out=x_sb, in_=x_d.rearrange("(k p) c -> p k c", p=P))
            L_sb = big.tile([P, KB, N], bf16, tag="L")
            for kb in range(KB):
                nc.sync.dma_start(out=L_sb[:, kb, :],
                                  in_=L_d[kb * P:(kb + 1) * P, :])
            if use_llo:
                Llo_sb = big.tile([P, KB, N], bf16, tag="Llo")
                for kb in range(KB):
                    nc.sync.dma_start(out=Llo_sb[:, kb, :],
                                      in_=Llo_d[kb * P:(kb + 1) * P, :])

            # state buffers (rotating) + accumulator + scratch
            ys = [state.tile([P, KB, DSH], f32, tag=f"y{i}", name=f"y{i}")
                  for i in range(3)]
            acc = state.tile([P, KB, DSH], f32, tag="acc")
            zh = state.tile([P, KB, DSH], f32, tag="zh")
            u = state.tile([P, KB, DSH], f32, tag="u")
            q = state.tile([P, KB, DSH], f32, tag="q")
            # double-buffered hi|lo moving operand: term k reads cats[k%2],
            # term k's splits write cats[(k+1)%2] (no WAR with own matmuls)
            cats = [state.tile([P, KB, 2 * DSH], bf16, tag=f"cat{i}",
                               name=f"cat{i}") for i in range(2)]

            sub = mybir.AluOpType.subtract
            add = mybir.AluOpType.add
            mult = mybir.AluOpType.mult

            SL = 4                 # vector-chain slices per term
            SKB = KB // SL

            def split_into_cat(src, cat, s):
                """cat slice <- [bf16(src) | bf16(src - hi)]"""
                sl = slice(s * SKB, (s + 1) * SKB)
                hi = cat[:, sl, 0:DSH]
                lo = cat[:, sl, DSH:2 * DSH]
                nc.scalar.copy(out=hi, in_=src[:, sl])
                nc.vector.scalar_tensor_tensor(out=lo, in0=hi, scalar=-1.0,
                                               in1=src[:, sl], op0=mult, op1=add)

            # y0 = x; acc = c0 * x
            nc.vector.tensor_copy(out=ys[0], in_=x_sb)
            nc.vector.tensor_scalar_mul(acc, x_sb, float(c[0]))
            for s in range(SL):
                split_into_cat(ys[0], cats[1], s)

            for k in range(1, K + 1):
                scale = float(2.0 * alpha) if k >= 2 else float(alpha)
                cat_r = cats[k % 2]
                cat_w = cats[(k + 1) % 2]
                y_cur = ys[(k - 1) % 3]
                y_next = ys[k % 3]
                ps = psum.tile([P, IB, 2 * DSH], f32, tag="ps")

                # q = -2*y_cur - y_prev (k>=2) or -y0 (k==1): ready before PSUM,
                # overlaps the matmul sweep
                for s in range(SL):
                    sl = slice(s * SKB, (s + 1) * SKB)
                    if k == 1:
                        nc.vector.tensor_scalar_mul(q[:, sl], y_cur[:, sl], -1.0)
                    else:
                        y_prev = ys[(k - 2) % 3]
                        nc.vector.scalar_tensor_tensor(
                            out=q[:, sl], in0=y_cur[:, sl], scalar=-2.0,
                            in1=y_prev[:, sl], op0=mult, op1=sub)

                for ib in range(IB):
                    for kb in range(KB):
                        nc.tensor.matmul(
                            ps[:, ib, :],
                            L_sb[:, kb, ib * P:(ib + 1) * P],
                            cat_r[:, kb, :],
                            start=(kb == 0),
                            stop=(kb == KB - 1 and not use_llo),
                        )
                    if use_llo:
                        for kb in range(KB):
                            nc.tensor.matmul(
                                ps[:, ib, 0:DSH],
                                Llo_sb[:, kb, ib * P:(ib + 1) * P],
                                cat_r[:, kb, 0:DSH],
                                start=False,
                                stop=(kb == KB - 1),
                            )

                for s in range(SL):
                    sl = slice(s * SKB, (s + 1) * SKB)
                    # zh = scale * ps_hi (ACT); u = scale * ps_lo + zh (DVE)
                    nc.scalar.mul(zh[:, sl], ps[:, sl, 0:DSH], scale)
                    nc.vector.scalar_tensor_tensor(
                        out=u[:, sl], in0=ps[:, sl, DSH:2 * DSH], scalar=scale,
                        in1=zh[:, sl], op0=mult, op1=add)
                    # y_next = u + q
                    nc.vector.tensor_add(out=y_next[:, sl], in0=u[:, sl],
                                         in1=q[:, sl])
                    if k < K:
                        split_into_cat(y_next, cat_w, s)
                    # acc += c_k * y_next (off critical path)
                    nc.vector.scalar_tensor_tensor(
                        out=acc[:, sl], in0=y_next[:, sl], scalar=float(c[k]),
                        in1=acc[:, sl], op0=mult, op1=add)

            nc.sync.dma_start(out=o_d.rearrange("(k p) c -> p k c", p=P), in_=acc)

    nc.compile()
    return nc


def kernel(x, L, t):
    x = np.ascontiguousarray(np.asarray(x, dtype=np.float32))
    L = np.ascontiguousarray(np.asarray(L, dtype=np.float32))
    tv = float(max(float(np.asarray(t, dtype=np.float32)), 1e-8))
    assert x.shape == (N, D) and L.shape == (N, N)

    lam_b = max(2.0 * float(np.diagonal(L).max()), 1e-6)
    alpha = 2.0 / lam_b
    c = _cheb_coeffs(tv, lam_b)

    L_hi = L.astype(BF16)
    L_res = L - L_hi.astype(np.float32)
    use_llo = bool(np.any(L_res != 0.0))

    nc = _build(tuple(float(v) for v in c), float(alpha), use_llo)

    in_maps = []
    for core in range(NCORES):
        m = {"L": L_hi, "x": np.ascontiguousarray(x[:, core * DSH:(core + 1) * DSH])}
        if use_llo:
            m["Llo"] = L_res.astype(BF16)
        in_maps.append(m)

    res = run_bass_kernel_spmd(nc, in_maps, core_ids=list(range(NCORES)))
    out = np.empty((N, D), dtype=np.float32)
    for core in range(NCORES):
        out[:, core * DSH:(core + 1) * DSH] = res.results[core]["out"]
    kernel.last_exec_time_ns = res.exec_time_ns
    kernel.last_results = res
    return out


kernel.last_exec_time_ns = None
kernel.last_results = None


# revision 13
# speedup vs baseline: 1.2836x; 1.1963x over previous
"""Heat-kernel graph diffusion on 8 Trainium2 NeuronCores.

Computes out = expm(-t*L) @ x for a graph Laplacian L [2048,2048] and node
features x [2048,512], t scalar.

Method: Chebyshev expansion of exp(-t*lam) on [0, lam_b] applied to the
action on x (no dense expm):
    out = sum_k c_k T_k(M) x,   M = (2/lam_b) L - I,
    c_0 = e^{-a} I_0(a), c_k = 2 e^{-a} (-1)^k I_k(a),  a = t*lam_b/2,
with lam_b = 2*max(diag(L)) (Gershgorin bound for a Laplacian; always
>= lam_max). K ~ 20 terms for t=0.5. Bessel I_k via Miller's backward
recurrence (pure numpy, no scipy).

Sharding: x column-sharded 8 ways (64 channels/core), L replicated; the
recurrence is embarrassingly parallel across channels - no collectives.

Device kernel (per core, natural layout [node, ch]):
  - L is exactly representable in bf16 (entries are multiples of 0.5 < 256),
    so it is passed pre-cast to bf16 and used as 128x128 stationary matmul
    weights (full PE array, 1 cyc/row). If a pathological L is not bf16-exact,
    a second bf16 matrix L_lo = L - bf16(L) is also multiplied in.
  - fp32 state y is split per term into bf16 hi+lo halves, concatenated as a
    [128, 128] moving operand; PSUM accumulates z_hi|z_lo in fp32.
  - Chebyshev recurrence y_next = 2a*(L y) - 2 y - y_prev and accumulation
    run in fp32 on the Vector/Scalar engines.
Measured end-to-end relative error vs the fp64 reference path: ~3e-5.
"""

import functools
import math

import numpy as np
import ml_dtypes

import concourse.bacc as bacc
import concourse.mybir as mybir
import concourse.tile as tile
from concourse.bass_utils import run_bass_kernel_spmd

N = 2048
D = 512
NCORES = 8
DSH = D // NCORES      # 64 channels per core
P = 128                # partitions
KB = N // P            # 16 contraction blocks
IB = N // P            # 16 output-row blocks
COEF_TOL = 3e-6
KMAX = 280

BF16 = np.dtype(ml_dtypes.bfloat16)


def _bessel_ive(nmax, a):
    """e^{-a} I_k(a), k=0..nmax, via Miller's backward recurrence (float64)."""
    if a < 1e-12:
        out = np.zeros(nmax + 1)
        out[0] = 1.0
        return out
    m = int(max(nmax, a) + 40 + 2 * math.sqrt(max(nmax, a)))
    r = np.zeros(m + 2)
    r[m] = 1e-300
    for k in range(m, 0, -1):
        r[k - 1] = r[k + 1] + (2.0 * k / a) * r[k]
        if r[k - 1] > 1e250:
            r /= r[k - 1]
    s = r[0] + 2.0 * np.sum(r[1:m + 1])
    return r[: nmax + 1] / s


def _cheb_coeffs(t, lam_b, tol=COEF_TOL, kcap=KMAX):
    a = t * lam_b / 2.0
    iv = _bessel_ive(kcap, a)
    c = np.empty(kcap + 1)
    c[0] = iv[0]
    c[1:] = 2.0 * iv[1:] * ((-1.0) ** np.arange(1, kcap + 1))
    keep = np.nonzero(np.abs(c) > tol)[0]
    K = max(1, int(keep[-1]) if len(keep) else 1)
    return c[: K + 1]


@functools.lru_cache(maxsize=4)
def _build(coeffs_key, alpha, use_llo):
    """Compile the per-core NEFF. coeffs_key: tuple of per-term float coeffs."""
    c = np.array(coeffs_key, dtype=np.float64)
    K = len(c) - 1
    f32 = mybir.dt.float32
    bf16 = mybir.dt.bfloat16

    nc = bacc.Bacc("TRN2", target_bir_lowering=False, debug=False,
                   num_devices=NCORES)
    L_d = nc.dram_tensor("L", [N, N], bf16, kind="ExternalInput").ap()
    Llo_d = None
    if use_llo:
        Llo_d = nc.dram_tensor("Llo", [N, N], bf16, kind="ExternalInput").ap()
    x_d = nc.dram_tensor("x", [N, DSH], f32, kind="ExternalInput").ap()
    o_d = nc.dram_tensor("out", [N, DSH], f32, kind="ExternalOutput").ap()

    with tile.TileContext(nc) as tc:
        with tc.tile_pool(name="big", bufs=1) as big, \
             tc.tile_pool(name="state", bufs=1) as state, \
             tc.tile_pool(name="psum", bufs=2, space="PSUM") as psum:
            x_sb = state.tile([P, KB, DSH], f32, tag="x")
            nc.sync.dma_start(out=x_sb, in_=x_d.rearrange("(k p) c -> p k c", p=P))
            L_sb = big.tile([P, KB, N], bf16, tag="L")
            for kb in range(KB):
                nc.sync.dma_start(out=L_sb[:, kb, :],
                                  in_=L_d[kb * P:(kb + 1) * P, :])
            if use_llo:
                Llo_sb = big.tile([P, KB, N], bf16, tag="Llo")
                for kb in range(KB):
                    nc.sync.dma_start(out=Llo_sb[:, kb, :],
                                      in_=Llo_d[kb * P:(kb + 1) * P, :])

            # state buffers (rotating) + accumulator + scratch
            ys = [state.tile([P, KB, DSH], f32, tag=f"y{i}", name=f"y{i}")
                  for i in range(3)]
            acc = state.tile([P, KB, DSH], f32, tag="acc")
            zh = state.tile([P, KB, DSH], f32, tag="zh")
            zh2 = state.tile([P, KB, DSH], f32, tag="zh2")
            u = state.tile([P, KB, DSH], f32, tag="u")
            u2 = state.tile([P, KB, DSH], f32, tag="u2")
            q = state.tile([P, KB, DSH], f32, tag="q")
            # double-buffered hi|lo moving operand: term k reads cats[k%2],
            # term k's splits write cats[(k+1)%2] (no WAR with own matmuls)
            cats = [state.tile([P, KB, 2 * DSH], bf16, tag=f"cat{i}",
                               name=f"cat{i}") for i in range(2)]

            sub = mybir.AluOpType.subtract
            add = mybir.AluOpType.add
            mult = mybir.AluOpType.mult

            SL = 4                 # vector-chain slices per term
            SKB = KB // SL

            def split_into_cat(src, cat, s):
                """cat slice <- [bf16(src) | bf16(src - hi)]"""
                sl = slice(s * SKB, (s + 1) * SKB)
                hi = cat[:, sl, 0:DSH]
                lo = cat[:, sl, DSH:2 * DSH]
                nc.scalar.copy(out=hi, in_=src[:, sl])
                nc.vector.scalar_tensor_tensor(out=lo, in0=hi, scalar=-1.0,
                                               in1=src[:, sl], op0=mult, op1=add)

            # y0 = x; acc = c0 * x
            nc.vector.tensor_copy(out=ys[0], in_=x_sb)
            nc.vector.tensor_scalar_mul(acc, x_sb, float(c[0]))
            for s in range(SL):
                split_into_cat(ys[0], cats[1], s)

            for k in range(1, K + 1):
                scale = float(2.0 * alpha) if k >= 2 else float(alpha)
                cat_r = cats[k % 2]
                cat_w = cats[(k + 1) % 2]
                y_cur = ys[(k - 1) % 3]
                y_next = ys[k % 3]
                # two PSUM regions, each a contiguous accumulation group:
                # ps1 sums kb 0..PH-1 (ready 75% into the term), ps2 sums the
                # tail kb. The next term's phase-1 matmuls only need ps1's
                # readers done, so the vector chain never stalls the PE.
                ps = psum.tile([P, IB, 2 * DSH], f32, tag="ps", bufs=1)
                ps2 = psum.tile([P, IB, 2 * DSH], f32, tag="ps2", bufs=1)

                # q = -2*y_cur - y_prev (k>=2) or -y0 (k==1): ready before PSUM,
                # overlaps the matmul sweep
                for s in range(SL):
                    sl = slice(s * SKB, (s + 1) * SKB)
                    if k == 1:
                        nc.vector.tensor_scalar_mul(q[:, sl], y_cur[:, sl], -1.0)
                    else:
                        y_prev = ys[(k - 2) % 3]
                        nc.vector.scalar_tensor_tensor(
                            out=q[:, sl], in0=y_cur[:, sl], scalar=-2.0,
                            in1=y_prev[:, sl], op0=mult, op1=sub)

                # two-phase contraction: phase 1 (kb 0..PH-1) only needs the
                # early cat slices, so it can start while the previous term's
                # tail slices are still in the vector chain -> no PE bubble
                PH = KB - SKB
                for tgt, lo_kb, hi_kb in ((ps, 0, PH), (ps2, PH, KB)):
                    for ib in range(IB):
                        for kb in range(lo_kb, hi_kb):
                            nc.tensor.matmul(
                                tgt[:, ib, :],
                                L_sb[:, kb, ib * P:(ib + 1) * P],
                                cat_r[:, kb, :],
                                start=(kb == lo_kb),
                                stop=(kb == hi_kb - 1),
                            )
                            if use_llo:
                                # correction term L_lo @ y_hi summed into the
                                # hi half (the chain adds both halves anyway)
                                nc.tensor.matmul(
                                    tgt[:, ib, 0:DSH],
                                    Llo_sb[:, kb, ib * P:(ib + 1) * P],
                                    cat_r[:, kb, 0:DSH],
                                    start=False,
                                    stop=(kb == hi_kb - 1),
                                    skip_group_check=True,
                                )

                for s in range(SL):
                    sl = slice(s * SKB, (s + 1) * SKB)
                    # combine both psum regions' hi|lo halves, scaled:
                    # u = scale*(ps.hi + ps.lo), u2 = scale*(ps2.hi+ps2.lo) + q
                    nc.scalar.mul(zh[:, sl], ps[:, sl, 0:DSH], scale)
                    nc.vector.scalar_tensor_tensor(
                        out=u[:, sl], in0=ps[:, sl, DSH:2 * DSH], scalar=scale,
                        in1=zh[:, sl], op0=mult, op1=add)
                    nc.scalar.mul(zh2[:, sl], ps2[:, sl, 0:DSH], scale)
                    nc.vector.scalar_tensor_tensor(
                        out=u2[:, sl], in0=ps2[:, sl, DSH:2 * DSH], scalar=scale,
                        in1=zh2[:, sl], op0=mult, op1=add)
                    nc.vector.tensor_add(out=u2[:, sl], in0=u2[:, sl],
                                         in1=q[:, sl])
                    # y_next = u + u2
                    nc.vector.tensor_add(out=y_next[:, sl], in0=u[:, sl],
                                         in1=u2[:, sl])
                    if k < K:
                        split_into_cat(y_next, cat_w, s)
                    # acc += c_k * y_next (off critical path)
                    nc.vector.scalar_tensor_tensor(
                        out=acc[:, sl], in0=y_next[:, sl], scalar=float(c[k]),
                        in1=acc[:, sl], op0=mult, op1=add)

            nc.sync.dma_start(out=o_d.rearrange("(k p) c -> p k c", p=P), in_=acc)

    nc.compile()
    return nc


def kernel(x, L, t):
    x = np.ascontiguousarray(np.asarray(x, dtype=np.float32))
    L = np.ascontiguousarray(np.asarray(L, dtype=np.float32))
    tv = float(max(float(np.asarray(t, dtype=np.float32)), 1e-8))
    assert x.shape == (N, D) and L.shape == (N, N)

    lam_b = max(2.0 * float(np.diagonal(L).max()), 1e-6)
    alpha = 2.0 / lam_b
    c = _cheb_coeffs(tv, lam_b)

    L_hi = L.astype(BF16)
    L_res = L - L_hi.astype(np.float32)
    use_llo = bool(np.any(L_res != 0.0))

    nc = _build(tuple(float(v) for v in c), float(alpha), use_llo)

    in_maps = []
    for core in range(NCORES):
        m = {"L": L_hi, "x": np.ascontiguousarray(x[:, core * DSH:(core + 1) * DSH])}
        if use_llo:
            m["Llo"] = L_res.astype(BF16)
        in_maps.append(m)

    res = run_bass_kernel_spmd(nc, in_maps, core_ids=list(range(NCORES)))
    out = np.empty((N, D), dtype=np.float32)
    for core in range(NCORES):
        out[:, core * DSH:(core + 1) * DSH] = res.results[core]["out"]
    kernel.last_exec_time_ns = res.exec_time_ns
    kernel.last_results = res
    return out


kernel.last_exec_time_ns = None
kernel.last_results = None
